# revision 34
# baseline (speedup 1.0000x reference)
"""DualRelGCN message-passing kernel for 8 TRN2 NeuronCores.

Strategy (destination-sharded, collective-free, X-stationary block-dense):
  - LayerNorm is invariant to positive per-row scaling, so LN(agg/denom) ==
    LN(agg): the denominator drops out entirely.
  - Shard edges by dst range: core c owns nodes [1250c, 1250(c+1)).  Each
    core computes its 1250 output rows locally -> no collectives.
  - agg^T[dim, dst] = sum_j X_j^T @ W_j: X pair-tiles (fp8, SBUF-resident)
    are the STATIONARY matmul operand; the dense weighted-adjacency blocks
    W_j (fp8, host-scattered) stream as the MOVING operand in 2-src-pair
    chunks.  Only 80 distinct stationaries (vs one per matmul): redundant
    compiler-emitted LDWEIGHTS are deleted post-compile, and redundant
    same-semaphore waits are elided so the PE issues back-to-back.
  - agg^T accumulates in PSUM across the whole sweep (10 fp32 [128,256]
    slices = 5 banks).  The sweep is DMA-bound on the 13.1MB W stream.
  - Epilogue exploits LN-affine/projection commutation:
        y = LN(agg) @ W^T = rstd*(agg @ W^T) - (mu*rstd)*(1 @ W^T)
    so no normalized tensor is ever materialized.  Row sums / sumsq come
    from N=1 matmuls against the already-loaded agg^T stationary; the
    per-row scalars for all 10 tiles are computed in one batched DVE pass;
    out = rel + alpha*y folds into two fused elementwise ops (DVE+Pool).
  - rel/out ride in bf16 (host casts) to halve their HBM traffic.
"""

import os
import sys

for _p in ("/opt/trn_rl_repo",):
    if _p not in sys.path:
        sys.path.insert(0, _p)

from contextlib import ExitStack

import numpy as np
import ml_dtypes

import concourse.bacc as bacc
import concourse.mybir as mybir
from concourse.alu_op_type import AluOpType
from concourse.tile import TileContext
from concourse.bass_utils import run_bass_kernel_spmd

F32 = mybir.dt.float32
BF16 = mybir.dt.bfloat16
FP8 = mybir.dt.float8e4
AF = mybir.ActivationFunctionType
DR = mybir.MatmulPerfMode.DoubleRow

N_NODES = 10000
DIM = 256
N_CORES = 8
NODES_PER_CORE = N_NODES // N_CORES  # 1250
TILE = 128
N_TILES = 10            # dst tiles per core (1280 rows incl. 30 pad)
OUT_ROWS = N_TILES * TILE
S_TILES = 80            # 79 src tiles + 1 zero pad -> even count
S_PAIRS = S_TILES // 2  # 40 DoubleRow pairs
N_CHUNKS = 10           # W streamed as 4-src-pair chunks
N_GROUPS = 5            # dst column groups of 256 (moving free dim limit)
ALPHA = 0.1
LN_EPS = 1e-5

_CACHE: dict = {}


def _dedup_ldweights(nc):
    """Delete InstLdweights whose AP matches the previously loaded one.

    The compiler emits one Ldweights per matmul; when consecutive matmuls
    share a stationary operand the PE array already holds it.  Only drops
    duplicates that carry no semaphore waits/updates.
    """
    removed = 0
    for fn in nc.m.functions:
        for blk in fn.blocks:
            insts = blk.instructions
            last_sig = None
            keep = []
            changed = False
            for i in insts:
                tn = type(i).__name__
                if tn == "InstLdweights":
                    si = i.sync_info
                    clean = si is None or (
                        len(si.on_wait) == 0 and len(si.on_update) == 0)
                    sig = (str(i.ins[0]), str(i.perf_mode),
                           str(i.is_transpose))
                    if clean and sig == last_sig:
                        removed += 1
                        changed = True
                        continue
                    last_sig = sig
                elif tn == "InstMatmult":
                    if getattr(i, "is_transpose", False):
                        last_sig = None
                elif tn in ("InstEventSemaphore", "InstDrain", "InstNop",
                            "InstNotify"):
                    pass  # does not clobber the PE weight array
                elif tn == "InstMatmultMx":
                    last_sig = None
                keep.append(i)
            if changed:
                while len(insts):
                    insts.pop()
                for i in keep:
                    insts.append(i)
    return removed


def _elide_redundant_waits(nc):
    """Drop semaphore waits already satisfied by an earlier wait on the
    same engine (same sem, >= value).  Sem counters only increment, so once
    an engine has waited for (s >= v), any later wait (s >= v' <= v) on
    that engine is a no-op.  Tile's own optimize_sems pass is disabled
    upstream; this narrow version only ever compares identical sems."""
    import bass_rust
    dropped = 0
    for fn in nc.m.functions:
        for blk in fn.blocks:
            seen: dict = {}  # (engine, sem_id) -> max value waited
            insts = blk.instructions
            keep = []
            changed = False
            for i in insts:
                si = i.sync_info
                eng = getattr(i, "engine", None)
                if si is not None and len(si.on_wait) > 0 and eng is not None:
                    new_waits = []
                    for w in si.on_wait:
                        if (getattr(w, "wait_mode", None) == "sem-ge-imm"
                                and getattr(w, "wait_reg", None) is None):
                            key = (str(eng), w.id)
                            v = w.wait_value
                            if seen.get(key, -1) >= v:
                                dropped += 1
                                continue
                            seen[key] = max(seen.get(key, -1), v)
                        new_waits.append(w)
                    if len(new_waits) != len(si.on_wait):
                        changed = True
                        if (len(new_waits) == 0 and len(si.on_update) == 0
                                and type(i).__name__ == "InstEventSemaphore"):
                            continue  # whole instruction is now a no-op
                        si.on_wait = new_waits
                keep.append(i)
            if changed:
                while len(insts):
                    insts.pop()
                for i in keep:
                    insts.append(i)
    return dropped


def _build():
    nc = bacc.Bacc("TRN2", target_bir_lowering=False, debug=False,
                   num_devices=N_CORES)

    x_d = nc.dram_tensor("x", [128, S_TILES * DIM], FP8,
                         kind="ExternalInput")
    w_d = nc.dram_tensor("wblk", [N_CHUNKS, 128, 8 * OUT_ROWS], FP8,
                         kind="ExternalInput")
    rel_d = nc.dram_tensor("relslice", [OUT_ROWS, DIM], BF16,
                           kind="ExternalInput")
    pwt_d = nc.dram_tensor("projwT", [128, 2 * DIM], BF16,
                           kind="ExternalInput")
    urep_d = nc.dram_tensor("urep", [128, DIM], BF16, kind="ExternalInput")
    out_d = nc.dram_tensor("out", [OUT_ROWS, DIM], BF16,
                           kind="ExternalOutput")

    with TileContext(nc) as tc, ExitStack() as es:
        const_pool = es.enter_context(tc.tile_pool(name="const", bufs=1))
        wpool = es.enter_context(tc.tile_pool(name="wblk", bufs=N_CHUNKS))
        ep_pool = es.enter_context(tc.tile_pool(name="ep", bufs=3))
        ps_agg = es.enter_context(tc.tile_pool(name="ps_agg", bufs=1,
                                               space="PSUM"))
        ps_rs = es.enter_context(tc.tile_pool(name="ps_rs", bufs=1,
                                              space="PSUM"))

        # --- constants / resident inputs ---
        # All W chunks are issued upfront on the sync ring (the whole W
        # stream fits in SBUF), with X chunk 0 wedged in right after W0 so
        # the sweep starts immediately; PE then runs behind the DMA
        # wavefront with no buffer-recycling waits.  Epilogue-only tensors
        # (pwt/urep/rel) ride the scalar ring after the X chunks.
        x0_sb = const_pool.tile([128, 20, DIM], FP8, tag="x0")
        x1_sb = const_pool.tile([128, S_TILES - 20, DIM], FP8, tag="x1")
        w_tiles = []
        for jj in range(N_CHUNKS):
            w4 = wpool.tile([128, 4, 2, OUT_ROWS], FP8, tag="w")
            w_tiles.append(w4)
        nc.sync.dma_start(w_tiles[0][:], w_d[0])
        nc.sync.dma_start(x0_sb[:], x_d[:, 0:20 * DIM])
        for jj in range(1, N_CHUNKS):
            nc.sync.dma_start(w_tiles[jj][:], w_d[jj])

        def x_pair(j, h):
            if 2 * j < 20:
                return x0_sb[:, 2 * j:2 * j + 2, 128 * h:128 * (h + 1)]
            return x1_sb[:, 2 * j - 20:2 * j - 18, 128 * h:128 * (h + 1)]

        epsb = const_pool.tile([128, 1], F32, tag="epsb")
        nc.vector.memset(epsb[:], LN_EPS)
        ones_col = const_pool.tile([128, 1], BF16, tag="ones")
        nc.vector.memset(ones_col[:], 1.0)
        nc.scalar.dma_start(x1_sb[:], x_d[:, 20 * DIM:S_TILES * DIM])
        pwt_sb = const_pool.tile([128, 2, DIM], BF16, tag="pwt")
        nc.scalar.dma_start(pwt_sb[:], pwt_d[:])
        urep_sb = const_pool.tile([128, DIM], BF16, tag="urep")
        nc.scalar.dma_start(urep_sb[:], urep_d[:])
        rel_sb = const_pool.tile([128, N_TILES, DIM], BF16, tag="rel")
        nc.scalar.dma_start(
            rel_sb[:],
            rel_d[:].rearrange("(t p) d -> p t d", t=N_TILES, p=128))

        # --- src sweep: agg^T accumulates in PSUM --------------------------
        # slot (h, g): dims [128h, 128h+128) x dst cols [256g, 256g+256)
        aggT = ps_agg.tile([128, 2 * N_GROUPS, 256], F32, tag="aggT")
        for jj in range(N_CHUNKS):
            w4 = w_tiles[jj]
            for a in range(4):
                j = 4 * jj + a
                for h in range(2):
                    lhs = x_pair(j, h)
                    for g in range(N_GROUPS):
                        nc.tensor.matmul(
                            aggT[:, h * N_GROUPS + g, :],
                            lhs,
                            w4[:, a, :, 256 * g:256 * (g + 1)],
                            start=(j == 0), stop=(j == S_PAIRS - 1),
                            perf_mode=DR)

        # --- epilogue ------------------------------------------------------
        # S1a: evacuate agg^T to SBUF bf16, one [128,256] copy per (h,g)
        # slot, split across ACT and DVE.
        aggT_sb = const_pool.tile([128, 2, N_TILES, 128], BF16, tag="aggTsb")
        for g in range(N_GROUPS):
            nc.scalar.copy(aggT_sb[:, 0, 2 * g:2 * g + 2, :],
                           aggT[:, 0 * N_GROUPS + g, :])
            nc.vector.tensor_copy(aggT_sb[:, 1, 2 * g:2 * g + 2, :],
                                  aggT[:, 1 * N_GROUPS + g, :])

        # S1b per tile: squares (DVE); then against the agg^T stationary
        # (loaded once per (t,k)): z = agg @ W^T into the freed PSUM banks
        # (generation 2 of the aggT tag) and rowsum via an N=1 matmul.
        # rs_all[:, t, 0] = sum_f agg^T, [:, t, 1] = sum_f (agg^T)^2.
        z_all = ps_agg.tile([128, 2 * N_GROUPS, 256], F32, tag="aggT")
        rs_all = ps_rs.tile([128, N_TILES, 2], F32, tag="rs")
        for t in range(N_TILES):
            sq = ep_pool.tile([128, 2, 128], BF16, tag="sq")
            nc.vector.tensor_tensor(sq[:], aggT_sb[:, :, t, :],
                                    aggT_sb[:, :, t, :], AluOpType.mult)
            for k in range(2):
                nc.tensor.matmul(rs_all[:, t, 0:1], aggT_sb[:, k, t, :],
                                 ones_col[:], start=(k == 0), stop=(k == 1))
                nc.tensor.matmul(z_all[:, t, :], aggT_sb[:, k, t, :],
                                 pwt_sb[:, k, :], start=(k == 0),
                                 stop=(k == 1))
            for k in range(2):
                nc.tensor.matmul(rs_all[:, t, 1:2], sq[:, k, :],
                                 ones_col[:], start=(k == 0), stop=(k == 1))

        # S2: batched per-row scalars for all tiles in one [128, 10] pass.
        #   mu = rs/256; var = ss/256 - mu^2; rstd = 1/sqrt(var + eps)
        #   s = alpha*rstd;  tcoef = -(alpha/256)*rs*rstd
        rs_sb = ep_pool.tile([128, N_TILES, 2], F32, tag="rs_sb")
        nc.vector.tensor_copy(rs_sb[:], rs_all[:])
        rs_f = rs_sb[:, :, 0]
        ss_f = rs_sb[:, :, 1]
        q = ep_pool.tile([128, N_TILES], F32, tag="q")
        nc.vector.tensor_tensor(q[:], rs_f, rs_f, AluOpType.mult)
        q2 = ep_pool.tile([128, N_TILES], F32, tag="q2")
        nc.vector.tensor_scalar(q2[:], q[:], -1.0 / (DIM * DIM), None,
                                AluOpType.mult)
        v = ep_pool.tile([128, N_TILES], F32, tag="v")
        nc.vector.scalar_tensor_tensor(v[:], ss_f, 1.0 / DIM, q2[:],
                                       AluOpType.mult, AluOpType.add)
        std = ep_pool.tile([128, N_TILES], F32, tag="std")
        nc.scalar.activation(std[:], v[:], AF.Sqrt, bias=epsb[:])
        rstd = ep_pool.tile([128, N_TILES], F32, tag="rstd")
        nc.vector.reciprocal(rstd[:], std[:])
        s_all = ep_pool.tile([128, N_TILES], F32, tag="s_all")
        nc.vector.tensor_scalar(s_all[:], rstd[:], ALPHA, None,
                                AluOpType.mult)
        t0 = ep_pool.tile([128, N_TILES], F32, tag="t0")
        nc.vector.tensor_tensor(t0[:], rs_f, rstd[:], AluOpType.mult)
        t_all = ep_pool.tile([128, N_TILES], F32, tag="t_all")
        nc.vector.tensor_scalar(t_all[:], t0[:], -ALPHA / DIM, None,
                                AluOpType.mult)

        # S3 per tile: out = rel + s*z + tcoef*u, split ACT / DVE / Pool:
        #   m2 = s*z (ACT, per-partition scale), acc = tcoef*u + rel (DVE),
        #   out = acc + m2 (Pool); one batched store on the idle sync ring.
        out_sb = const_pool.tile([128, N_TILES, DIM], BF16, tag="outsb")
        for t in range(N_TILES):
            m2 = ep_pool.tile([128, DIM], BF16, tag="m2")
            nc.scalar.activation(m2[:], z_all[:, t, :], AF.Copy,
                                 scale=s_all[:, t:t + 1])
            acc = ep_pool.tile([128, DIM], BF16, tag="acc")
            nc.vector.scalar_tensor_tensor(acc[:], urep_sb[:],
                                           t_all[:, t:t + 1],
                                           rel_sb[:, t, :],
                                           AluOpType.mult, AluOpType.add)
            nc.gpsimd.tensor_add(out_sb[:, t, :], acc[:], m2[:])
        nc.sync.dma_start(
            out_d[:].rearrange("(t p) d -> p t d", t=N_TILES, p=128),
            out_sb[:])

    nc.compile()
    if os.environ.get("NO_DEDUP") != "1":
        n1 = 0
        if os.environ.get("NO_LDW_DEDUP") != "1":
            n1 = _dedup_ldweights(nc)
        n2 = 0
        if os.environ.get("DO_ELIDE") == "1":
            n2 = _elide_redundant_waits(nc)
        print(f"[kernel] dedup ldweights: {n1}, elided waits: {n2}")
    if os.environ.get("DUMP_IR"):
        with open("/root/problem/work/ir_dump.txt", "w") as f:
            for fn in nc.m.functions:
                for bi, blk in enumerate(fn.blocks):
                    f.write(f"== block {bi} ==\n")
                    for i in blk.instructions:
                        si = i.sync_info
                        w_ = ([f"{w.ant_name}>={w.wait_value}"
                               for w in si.on_wait] if si else [])
                        u_ = ([f"{u.ant_name}+={u.update_value}"
                               for u in si.on_update] if si else [])
                        f.write(f"{type(i).__name__:24s} eng={i.engine} "
                                f"wait={w_} upd={u_}\n")
    return nc


def _prep(rel_embed, rel_edge_index, rel_edge_weight, proj_w):
    """Host-side sharding/layout: scatter edges into dense per-(src pair)
    weight blocks; lay out rel_embed for SBUF residency."""
    src = np.asarray(rel_edge_index[0], dtype=np.int64)
    dst = np.asarray(rel_edge_index[1], dtype=np.int64)
    w = np.asarray(rel_edge_weight, dtype=np.float32)
    rel = np.asarray(rel_embed, dtype=np.float32)
    pw = np.asarray(proj_w, dtype=np.float32)

    core = dst // NODES_PER_CORE
    drel = dst - core * NODES_PER_CORE
    t = drel // TILE
    d = drel % TILE
    s = src // TILE
    p = src % TILE
    # flat index inside one core's [N_TILES, S_TILES, 128, 128] block array
    flat = ((t * S_TILES + s) * TILE + p) * TILE + d
    blk_sz = N_TILES * S_TILES * TILE * TILE

    w_dev = np.empty((N_CORES, N_CHUNKS, 128, 8 * OUT_ROWS),
                     dtype=ml_dtypes.float8_e4m3)
    for c in range(N_CORES):
        m = core == c
        wc = np.bincount(flat[m], weights=w[m], minlength=blk_sz)
        wc = wc.reshape(N_TILES, S_TILES, TILE, TILE).astype(np.float32)
        # [t, s, p, d] -> [jj, p, (a, q, t*128+d)]
        arr = wc.transpose(1, 2, 0, 3).reshape(S_TILES, 128, OUT_ROWS)
        arr = arr.reshape(N_CHUNKS, 8, 128, OUT_ROWS).transpose(0, 2, 1, 3)
        w_dev[c] = arr.reshape(N_CHUNKS, 128, 8 * OUT_ROWS)

    rel8 = rel.astype(ml_dtypes.float8_e4m3)
    rel8_pad = np.zeros((S_TILES * TILE, DIM), dtype=ml_dtypes.float8_e4m3)
    rel8_pad[:N_NODES] = rel8
    x_dev = np.ascontiguousarray(
        rel8_pad.reshape(S_TILES, TILE, DIM).transpose(1, 0, 2).reshape(
            128, S_TILES * DIM))

    relslice = np.zeros((N_CORES, OUT_ROWS, DIM), dtype=ml_dtypes.bfloat16)
    for c in range(N_CORES):
        relslice[c, :NODES_PER_CORE] = rel[c * NODES_PER_CORE:
                                           (c + 1) * NODES_PER_CORE]
    pwt = pw.T.astype(np.float32)  # [f, o]
    pwt_dev = np.ascontiguousarray(
        pwt.reshape(2, 128, DIM).transpose(1, 0, 2).reshape(
            128, 2 * DIM)).astype(ml_dtypes.bfloat16)
    # u[o] = sum_f W[o, f], replicated across partitions
    u = pw.sum(axis=1).astype(np.float32)
    urep = np.broadcast_to(u, (128, DIM)).astype(ml_dtypes.bfloat16)
    urep = np.ascontiguousarray(urep)

    in_maps = []
    for c in range(N_CORES):
        in_maps.append({
            "x": x_dev,
            "wblk": w_dev[c],
            "relslice": relslice[c],
            "projwT": pwt_dev,
            "urep": urep,
        })
    return in_maps


def kernel(rel_embed, rel_edge_index, rel_edge_weight, proj_w,
           _trace=False):
    in_maps = _prep(rel_embed, rel_edge_index, rel_edge_weight, proj_w)
    nc = _CACHE.get("nc")
    if nc is None:
        nc = _build()
        _CACHE["nc"] = nc
    res = run_bass_kernel_spmd(nc, in_maps, core_ids=list(range(N_CORES)),
                               trace=_trace)
    out = np.concatenate(
        [res.results[c]["out"][:NODES_PER_CORE] for c in range(N_CORES)],
        axis=0)
    if _trace:
        kernel.last_results = res
    return out.astype(np.float32)


# revision 35
# speedup vs baseline: 1.0546x; 1.0546x over previous
"""DualRelGCN message-passing kernel for 8 TRN2 NeuronCores.

Strategy (destination-sharded, collective-free, X-stationary block-dense):
  - LayerNorm is invariant to positive per-row scaling, so LN(agg/denom) ==
    LN(agg): the denominator drops out entirely.
  - Shard edges by dst range: core c owns nodes [1250c, 1250(c+1)).  Each
    core computes its 1250 output rows locally -> no collectives.
  - agg^T[dim, dst] = sum_j X_j^T @ W_j: X pair-tiles (fp8, SBUF-resident)
    are the STATIONARY matmul operand; the dense weighted-adjacency blocks
    W_j (fp8, host-scattered) stream as the MOVING operand in 2-src-pair
    chunks.  Only 80 distinct stationaries (vs one per matmul): redundant
    compiler-emitted LDWEIGHTS are deleted post-compile, and redundant
    same-semaphore waits are elided so the PE issues back-to-back.
  - agg^T accumulates in PSUM across the whole sweep (10 fp32 [128,256]
    slices = 5 banks).  The sweep is DMA-bound on the 13.1MB W stream.
  - Epilogue exploits LN-affine/projection commutation:
        y = LN(agg) @ W^T = rstd*(agg @ W^T) - (mu*rstd)*(1 @ W^T)
    so no normalized tensor is ever materialized.  Row sums / sumsq come
    from N=1 matmuls against the already-loaded agg^T stationary; the
    per-row scalars for all 10 tiles are computed in one batched DVE pass;
    out = rel + alpha*y folds into two fused elementwise ops (DVE+Pool).
  - rel/out ride in bf16 (host casts) to halve their HBM traffic.
"""

import os
import sys

for _p in ("/opt/trn_rl_repo",):
    if _p not in sys.path:
        sys.path.insert(0, _p)

from contextlib import ExitStack

import numpy as np
import ml_dtypes

import concourse.bacc as bacc
import concourse.mybir as mybir
from concourse.alu_op_type import AluOpType
from concourse.tile import TileContext
from concourse.bass_utils import run_bass_kernel_spmd

F32 = mybir.dt.float32
BF16 = mybir.dt.bfloat16
FP8 = mybir.dt.float8e4
AF = mybir.ActivationFunctionType
DR = mybir.MatmulPerfMode.DoubleRow

N_NODES = 10000
DIM = 256
N_CORES = 8
NODES_PER_CORE = N_NODES // N_CORES  # 1250
TILE = 128
N_TILES = 10            # dst tiles per core (1280 rows incl. 30 pad)
OUT_ROWS = N_TILES * TILE
S_TILES = 80            # 79 src tiles + 1 zero pad -> even count
S_PAIRS = S_TILES // 2  # 40 DoubleRow pairs
N_CHUNKS = 10           # W streamed as 4-src-pair chunks
N_GROUPS = 5            # dst column groups of 256 (moving free dim limit)
ALPHA = 0.1
LN_EPS = 1e-5

_CACHE: dict = {}


def _dedup_ldweights(nc):
    """Delete InstLdweights whose AP matches the previously loaded one.

    The compiler emits one Ldweights per matmul; when consecutive matmuls
    share a stationary operand the PE array already holds it.  Only drops
    duplicates that carry no semaphore waits/updates.
    """
    removed = 0
    for fn in nc.m.functions:
        for blk in fn.blocks:
            insts = blk.instructions
            last_sig = None
            keep = []
            changed = False
            for i in insts:
                tn = type(i).__name__
                if tn == "InstLdweights":
                    si = i.sync_info
                    clean = si is None or (
                        len(si.on_wait) == 0 and len(si.on_update) == 0)
                    sig = (str(i.ins[0]), str(i.perf_mode),
                           str(i.is_transpose))
                    if clean and sig == last_sig:
                        removed += 1
                        changed = True
                        continue
                    last_sig = sig
                elif tn == "InstMatmult":
                    if getattr(i, "is_transpose", False):
                        last_sig = None
                elif tn in ("InstEventSemaphore", "InstDrain", "InstNop",
                            "InstNotify"):
                    pass  # does not clobber the PE weight array
                elif tn == "InstMatmultMx":
                    last_sig = None
                keep.append(i)
            if changed:
                while len(insts):
                    insts.pop()
                for i in keep:
                    insts.append(i)
    return removed


def _elide_redundant_waits(nc):
    """Drop semaphore waits already satisfied by an earlier wait on the
    same engine (same sem, >= value).  Sem counters only increment, so once
    an engine has waited for (s >= v), any later wait (s >= v' <= v) on
    that engine is a no-op.  Tile's own optimize_sems pass is disabled
    upstream; this narrow version only ever compares identical sems."""
    import bass_rust
    dropped = 0
    for fn in nc.m.functions:
        for blk in fn.blocks:
            seen: dict = {}  # (engine, sem_id) -> max value waited
            insts = blk.instructions
            keep = []
            changed = False
            for i in insts:
                si = i.sync_info
                eng = getattr(i, "engine", None)
                if si is not None and len(si.on_wait) > 0 and eng is not None:
                    new_waits = []
                    for w in si.on_wait:
                        if (getattr(w, "wait_mode", None) == "sem-ge-imm"
                                and getattr(w, "wait_reg", None) is None):
                            key = (str(eng), w.id)
                            v = w.wait_value
                            if seen.get(key, -1) >= v:
                                dropped += 1
                                continue
                            seen[key] = max(seen.get(key, -1), v)
                        new_waits.append(w)
                    if len(new_waits) != len(si.on_wait):
                        changed = True
                        if (len(new_waits) == 0 and len(si.on_update) == 0
                                and type(i).__name__ == "InstEventSemaphore"):
                            continue  # whole instruction is now a no-op
                        si.on_wait = new_waits
                keep.append(i)
            if changed:
                while len(insts):
                    insts.pop()
                for i in keep:
                    insts.append(i)
    return dropped


def _build():
    nc = bacc.Bacc("TRN2", target_bir_lowering=False, debug=False,
                   num_devices=N_CORES)

    x_d = nc.dram_tensor("x", [128, S_TILES * DIM], FP8,
                         kind="ExternalInput")
    w_d = nc.dram_tensor("wblk", [N_CHUNKS, 128, 8 * OUT_ROWS], FP8,
                         kind="ExternalInput")
    rel_d = nc.dram_tensor("relslice", [OUT_ROWS, DIM], BF16,
                           kind="ExternalInput")
    pwt_d = nc.dram_tensor("projwT", [128, 2 * DIM], BF16,
                           kind="ExternalInput")
    urep_d = nc.dram_tensor("urep", [128, DIM], BF16, kind="ExternalInput")
    out_d = nc.dram_tensor("out", [OUT_ROWS, DIM], BF16,
                           kind="ExternalOutput")

    with TileContext(nc) as tc, ExitStack() as es:
        const_pool = es.enter_context(tc.tile_pool(name="const", bufs=1))
        wpool = es.enter_context(tc.tile_pool(name="wblk", bufs=N_CHUNKS))
        ep_pool = es.enter_context(tc.tile_pool(name="ep", bufs=3))
        ps_agg = es.enter_context(tc.tile_pool(name="ps_agg", bufs=1,
                                               space="PSUM"))
        ps_rs = es.enter_context(tc.tile_pool(name="ps_rs", bufs=1,
                                              space="PSUM"))

        # --- constants / resident inputs ---
        # All W chunks are issued upfront on the sync ring (the whole W
        # stream fits in SBUF), with X chunk 0 wedged in right after W0 so
        # the sweep starts immediately; PE then runs behind the DMA
        # wavefront with no buffer-recycling waits.  Epilogue-only tensors
        # (pwt/urep/rel) ride the scalar ring after the X chunks.
        x0_sb = const_pool.tile([128, 20, DIM], FP8, tag="x0")
        x1_sb = const_pool.tile([128, S_TILES - 20, DIM], FP8, tag="x1")
        w_tiles = []
        for jj in range(N_CHUNKS):
            w4 = wpool.tile([128, 4, 2, OUT_ROWS], FP8, tag="w")
            w_tiles.append(w4)
        nc.sync.dma_start(w_tiles[0][:], w_d[0])
        nc.sync.dma_start(x0_sb[:], x_d[:, 0:20 * DIM])
        for jj in range(1, N_CHUNKS):
            nc.sync.dma_start(w_tiles[jj][:], w_d[jj])

        def x_pair(j, h):
            if 2 * j < 20:
                return x0_sb[:, 2 * j:2 * j + 2, 128 * h:128 * (h + 1)]
            return x1_sb[:, 2 * j - 20:2 * j - 18, 128 * h:128 * (h + 1)]

        epsb = const_pool.tile([128, 1], F32, tag="epsb")
        nc.vector.memset(epsb[:], LN_EPS)
        ones_col = const_pool.tile([128, 1], BF16, tag="ones")
        nc.vector.memset(ones_col[:], 1.0)
        nc.scalar.dma_start(x1_sb[:], x_d[:, 20 * DIM:S_TILES * DIM])
        pwt_sb = const_pool.tile([128, 2, DIM], BF16, tag="pwt")
        nc.scalar.dma_start(pwt_sb[:], pwt_d[:])
        urep_sb = const_pool.tile([128, DIM], BF16, tag="urep")
        nc.scalar.dma_start(urep_sb[:], urep_d[:])
        rel_sb = const_pool.tile([128, N_TILES, DIM], BF16, tag="rel")
        nc.scalar.dma_start(
            rel_sb[:],
            rel_d[:].rearrange("(t p) d -> p t d", t=N_TILES, p=128))

        # --- src sweep: agg^T accumulates in PSUM --------------------------
        # slot (h, g): dims [128h, 128h+128) x dst cols [256g, 256g+256)
        aggT = ps_agg.tile([128, 2 * N_GROUPS, 256], F32, tag="aggT")
        for jj in range(N_CHUNKS):
            w4 = w_tiles[jj]
            for a in range(4):
                j = 4 * jj + a
                for h in range(2):
                    lhs = x_pair(j, h)
                    for g in range(N_GROUPS):
                        nc.tensor.matmul(
                            aggT[:, h * N_GROUPS + g, :],
                            lhs,
                            w4[:, a, :, 256 * g:256 * (g + 1)],
                            start=(j == 0), stop=(j == S_PAIRS - 1),
                            perf_mode=DR)

        # --- epilogue ------------------------------------------------------
        # S1a: evacuate agg^T to SBUF bf16, one [128,256] copy per (h,g)
        # slot, split across ACT and DVE.
        aggT_sb = const_pool.tile([128, 2, N_TILES, 128], BF16, tag="aggTsb")
        for g in range(N_GROUPS):
            nc.scalar.copy(aggT_sb[:, 0, 2 * g:2 * g + 2, :],
                           aggT[:, 0 * N_GROUPS + g, :])
            nc.vector.tensor_copy(aggT_sb[:, 1, 2 * g:2 * g + 2, :],
                                  aggT[:, 1 * N_GROUPS + g, :])

        # S1b per tile: squares (DVE); then against the agg^T stationary
        # (loaded once per (t,k)): z = agg @ W^T into the freed PSUM banks
        # (generation 2 of the aggT tag) and rowsum via an N=1 matmul.
        # rs_all[:, t, 0] = sum_f agg^T, [:, t, 1] = sum_f (agg^T)^2.
        z_all = ps_agg.tile([128, 2 * N_GROUPS, 256], F32, tag="aggT")
        rs_all = ps_rs.tile([128, N_TILES, 2], F32, tag="rs")
        for t in range(N_TILES):
            sq = ep_pool.tile([128, 2, 128], BF16, tag="sq")
            nc.vector.tensor_tensor(sq[:], aggT_sb[:, :, t, :],
                                    aggT_sb[:, :, t, :], AluOpType.mult)
            for k in range(2):
                nc.tensor.matmul(rs_all[:, t, 0:1], aggT_sb[:, k, t, :],
                                 ones_col[:], start=(k == 0), stop=(k == 1))
                nc.tensor.matmul(z_all[:, t, :], aggT_sb[:, k, t, :],
                                 pwt_sb[:, k, :], start=(k == 0),
                                 stop=(k == 1))
            for k in range(2):
                nc.tensor.matmul(rs_all[:, t, 1:2], sq[:, k, :],
                                 ones_col[:], start=(k == 0), stop=(k == 1))

        # S2: batched per-row scalars for all tiles in one [128, 10] pass.
        #   mu = rs/256; var = ss/256 - mu^2; rstd = 1/sqrt(var + eps)
        #   s = alpha*rstd;  tcoef = -(alpha/256)*rs*rstd
        rs_sb = ep_pool.tile([128, N_TILES, 2], F32, tag="rs_sb")
        nc.vector.tensor_copy(rs_sb[:], rs_all[:])
        rs_f = rs_sb[:, :, 0]
        ss_f = rs_sb[:, :, 1]
        q = ep_pool.tile([128, N_TILES], F32, tag="q")
        nc.vector.tensor_tensor(q[:], rs_f, rs_f, AluOpType.mult)
        q2 = ep_pool.tile([128, N_TILES], F32, tag="q2")
        nc.vector.tensor_scalar(q2[:], q[:], -1.0 / (DIM * DIM), None,
                                AluOpType.mult)
        v = ep_pool.tile([128, N_TILES], F32, tag="v")
        nc.vector.scalar_tensor_tensor(v[:], ss_f, 1.0 / DIM, q2[:],
                                       AluOpType.mult, AluOpType.add)
        std = ep_pool.tile([128, N_TILES], F32, tag="std")
        nc.scalar.activation(std[:], v[:], AF.Sqrt, bias=epsb[:])
        rstd = ep_pool.tile([128, N_TILES], F32, tag="rstd")
        nc.vector.reciprocal(rstd[:], std[:])
        s_all = ep_pool.tile([128, N_TILES], F32, tag="s_all")
        nc.vector.tensor_scalar(s_all[:], rstd[:], ALPHA, None,
                                AluOpType.mult)
        t0 = ep_pool.tile([128, N_TILES], F32, tag="t0")
        nc.vector.tensor_tensor(t0[:], rs_f, rstd[:], AluOpType.mult)
        t_all = ep_pool.tile([128, N_TILES], F32, tag="t_all")
        nc.vector.tensor_scalar(t_all[:], t0[:], -ALPHA / DIM, None,
                                AluOpType.mult)

        # S3 per tile: out = rel + s*z + tcoef*u, split ACT / DVE / Pool:
        #   m2 = s*z (ACT, per-partition scale), acc = tcoef*u + rel (DVE),
        #   out = acc + m2 (Pool); one batched store on the idle sync ring.
        out_sb = const_pool.tile([128, N_TILES, DIM], BF16, tag="outsb")
        for t in range(N_TILES):
            m2 = ep_pool.tile([128, DIM], BF16, tag="m2")
            nc.scalar.activation(m2[:], z_all[:, t, :], AF.Copy,
                                 scale=s_all[:, t:t + 1])
            acc = ep_pool.tile([128, DIM], BF16, tag="acc")
            nc.vector.scalar_tensor_tensor(acc[:], urep_sb[:],
                                           t_all[:, t:t + 1],
                                           rel_sb[:, t, :],
                                           AluOpType.mult, AluOpType.add)
            nc.vector.tensor_tensor(out_sb[:, t, :], acc[:], m2[:],
                                    AluOpType.add)
        nc.sync.dma_start(
            out_d[:].rearrange("(t p) d -> p t d", t=N_TILES, p=128),
            out_sb[:])

    nc.compile()
    if os.environ.get("NO_DEDUP") != "1":
        n1 = 0
        if os.environ.get("NO_LDW_DEDUP") != "1":
            n1 = _dedup_ldweights(nc)
        n2 = 0
        if os.environ.get("DO_ELIDE") == "1":
            n2 = _elide_redundant_waits(nc)
        print(f"[kernel] dedup ldweights: {n1}, elided waits: {n2}")
    if os.environ.get("DUMP_IR"):
        with open("/root/problem/work/ir_dump.txt", "w") as f:
            for fn in nc.m.functions:
                for bi, blk in enumerate(fn.blocks):
                    f.write(f"== block {bi} ==\n")
                    for i in blk.instructions:
                        si = i.sync_info
                        w_ = ([f"{w.ant_name}>={w.wait_value}"
                               for w in si.on_wait] if si else [])
                        u_ = ([f"{u.ant_name}+={u.update_value}"
                               for u in si.on_update] if si else [])
                        f.write(f"{type(i).__name__:24s} eng={i.engine} "
                                f"wait={w_} upd={u_}\n")
    return nc


def _prep(rel_embed, rel_edge_index, rel_edge_weight, proj_w):
    """Host-side sharding/layout: scatter edges into dense per-(src pair)
    weight blocks; lay out rel_embed for SBUF residency."""
    src = np.asarray(rel_edge_index[0], dtype=np.int64)
    dst = np.asarray(rel_edge_index[1], dtype=np.int64)
    w = np.asarray(rel_edge_weight, dtype=np.float32)
    rel = np.asarray(rel_embed, dtype=np.float32)
    pw = np.asarray(proj_w, dtype=np.float32)

    core = dst // NODES_PER_CORE
    drel = dst - core * NODES_PER_CORE
    t = drel // TILE
    d = drel % TILE
    s = src // TILE
    p = src % TILE
    # flat index inside one core's [N_TILES, S_TILES, 128, 128] block array
    flat = ((t * S_TILES + s) * TILE + p) * TILE + d
    blk_sz = N_TILES * S_TILES * TILE * TILE

    w_dev = np.empty((N_CORES, N_CHUNKS, 128, 8 * OUT_ROWS),
                     dtype=ml_dtypes.float8_e4m3)
    for c in range(N_CORES):
        m = core == c
        wc = np.bincount(flat[m], weights=w[m], minlength=blk_sz)
        wc = wc.reshape(N_TILES, S_TILES, TILE, TILE).astype(np.float32)
        # [t, s, p, d] -> [jj, p, (a, q, t*128+d)]
        arr = wc.transpose(1, 2, 0, 3).reshape(S_TILES, 128, OUT_ROWS)
        arr = arr.reshape(N_CHUNKS, 8, 128, OUT_ROWS).transpose(0, 2, 1, 3)
        w_dev[c] = arr.reshape(N_CHUNKS, 128, 8 * OUT_ROWS)

    rel8 = rel.astype(ml_dtypes.float8_e4m3)
    rel8_pad = np.zeros((S_TILES * TILE, DIM), dtype=ml_dtypes.float8_e4m3)
    rel8_pad[:N_NODES] = rel8
    x_dev = np.ascontiguousarray(
        rel8_pad.reshape(S_TILES, TILE, DIM).transpose(1, 0, 2).reshape(
            128, S_TILES * DIM))

    relslice = np.zeros((N_CORES, OUT_ROWS, DIM), dtype=ml_dtypes.bfloat16)
    for c in range(N_CORES):
        relslice[c, :NODES_PER_CORE] = rel[c * NODES_PER_CORE:
                                           (c + 1) * NODES_PER_CORE]
    pwt = pw.T.astype(np.float32)  # [f, o]
    pwt_dev = np.ascontiguousarray(
        pwt.reshape(2, 128, DIM).transpose(1, 0, 2).reshape(
            128, 2 * DIM)).astype(ml_dtypes.bfloat16)
    # u[o] = sum_f W[o, f], replicated across partitions
    u = pw.sum(axis=1).astype(np.float32)
    urep = np.broadcast_to(u, (128, DIM)).astype(ml_dtypes.bfloat16)
    urep = np.ascontiguousarray(urep)

    in_maps = []
    for c in range(N_CORES):
        in_maps.append({
            "x": x_dev,
            "wblk": w_dev[c],
            "relslice": relslice[c],
            "projwT": pwt_dev,
            "urep": urep,
        })
    return in_maps


def kernel(rel_embed, rel_edge_index, rel_edge_weight, proj_w,
           _trace=False):
    in_maps = _prep(rel_embed, rel_edge_index, rel_edge_weight, proj_w)
    nc = _CACHE.get("nc")
    if nc is None:
        nc = _build()
        _CACHE["nc"] = nc
    res = run_bass_kernel_spmd(nc, in_maps, core_ids=list(range(N_CORES)),
                               trace=_trace)
    out = np.concatenate(
        [res.results[c]["out"][:NODES_PER_CORE] for c in range(N_CORES)],
        axis=0)
    if _trace:
        kernel.last_results = res
    return out.astype(np.float32)


# revision 36
# speedup vs baseline: 1.1980x; 1.1361x over previous
"""DualRelGCN message-passing kernel for 8 TRN2 NeuronCores.

Strategy (destination-sharded, collective-free, block-dense):
  - LayerNorm is invariant to positive per-row scaling, so LN(agg/denom) ==
    LN(agg): the denominator drops out of the computation entirely.
  - Shard edges by dst range: core c owns nodes [1250c, 1250(c+1)) and
    receives every edge whose dst falls there.  Each core computes its 1250
    output rows completely locally -> no collectives.
  - The weighted gather+segment_sum is expressed as a block matmul:
    agg[tile t] = sum_s W_ts.T @ X_s, where W_ts is the [128 src, 128 dst]
    dense block of the weighted adjacency (host-scattered from the edge
    list; ~5% nnz but dense matmul on PE beats any descriptor-generated
    gather path by a wide margin) and X_s is a [128, 256] tile of rel_embed
    (bf16, fully resident in SBUF).  PSUM accumulates over s in fp32.
  - Epilogue per dst tile: LN on DVE/ACT, PE transpose, y = ln @ proj_w.T,
    out = rel_embed + 0.1*y.
  - The device program is fully static: the edge distribution only changes
    tensor *contents*, never the instruction stream.
"""

import sys

for _p in ("/opt/trn_rl_repo",):
    if _p not in sys.path:
        sys.path.insert(0, _p)

from contextlib import ExitStack

import numpy as np
import ml_dtypes

import concourse.bacc as bacc
import concourse.mybir as mybir
from concourse.alu_op_type import AluOpType
from concourse.tile import TileContext
from concourse.bass_utils import run_bass_kernel_spmd

F32 = mybir.dt.float32
BF16 = mybir.dt.bfloat16
FP8 = mybir.dt.float8e4
AF = mybir.ActivationFunctionType

N_NODES = 10000
DIM = 256
N_CORES = 8
NODES_PER_CORE = N_NODES // N_CORES  # 1250
TILE = 128
N_TILES = -(-NODES_PER_CORE // TILE)  # 10 dst tiles per core
S_TILES = -(-N_NODES // TILE) + 1  # 79 src tiles + 1 zero pad -> even count
S_PAIRS = S_TILES // 2  # DoubleRow matmuls contract two src tiles at once
OUT_ROWS = N_TILES * TILE  # 1280
ALPHA = 0.1
LN_EPS = 1e-5
# chunk boundaries (src-tile units): small leading chunks let the first
# matmuls start before the bulk of X/W lands
X_BOUNDS = [0, 20, 40, 60, 80]
XDIM = DIM
W_BOUNDS0 = [0, 20, 40, 60, 80]  # dst tile 0 (startup critical)
W_BOUNDS = [0, 20, 40, 60, 80]  # steady-state tiles

_CACHE: dict = {}


def _build():
    nc = bacc.Bacc("TRN2", target_bir_lowering=False, debug=False,
                   num_devices=N_CORES)

    x_d = nc.dram_tensor("x", [128, S_TILES * XDIM], FP8,
                        kind="ExternalInput")
    w_d = nc.dram_tensor("wblk", [N_TILES, 128, S_TILES * TILE], FP8,
                         kind="ExternalInput")
    relsl_d = nc.dram_tensor("relslice", [OUT_ROWS, DIM], F32,
                             kind="ExternalInput")
    pwt_d = nc.dram_tensor("projwT", [128, 2 * DIM], BF16,
                           kind="ExternalInput")
    out_d = nc.dram_tensor("out", [OUT_ROWS, DIM], F32, kind="ExternalOutput")

    with TileContext(nc) as tc, ExitStack() as es:
        const_pool = es.enter_context(tc.tile_pool(name="const", bufs=1))
        wpool = es.enter_context(tc.tile_pool(name="wblk", bufs=4))
        ep_pool = es.enter_context(tc.tile_pool(name="ep", bufs=2))
        ps_agg = es.enter_context(tc.tile_pool(name="ps_agg", bufs=3,
                                               space="PSUM"))
        ps_tr = es.enter_context(tc.tile_pool(name="ps_tr", bufs=3,
                                              space="PSUM"))
        ps_y = es.enter_context(tc.tile_pool(name="ps_y", bufs=2,
                                             space="PSUM"))

        # --- constants / resident inputs ---
        iota_row = const_pool.tile([128, 128], F32, tag="iota")
        nc.gpsimd.iota(iota_row[:], [[1, 128]], base=0, channel_multiplier=0,
                       allow_small_or_imprecise_dtypes=True)
        pidx = const_pool.tile([128, 1], F32, tag="pidx")
        nc.gpsimd.iota(pidx[:], [[1, 1]], base=0, channel_multiplier=1,
                       allow_small_or_imprecise_dtypes=True)
        ident = const_pool.tile([128, 128], BF16, tag="ident")
        nc.vector.tensor_scalar(ident[:], iota_row[:], pidx[:], None,
                                AluOpType.is_equal)
        epsb = const_pool.tile([128, 1], F32, tag="epsb")
        nc.vector.memset(epsb[:], LN_EPS)
        pwt_sb = const_pool.tile([128, 2, DIM], BF16, tag="pwt")
        nc.scalar.dma_start(pwt_sb[:], pwt_d[:])  # scalar: off W's ring

        # rel_embed (fp8), fully resident; chunked load so dst-tile 0's
        # matmuls can start before the whole stream lands.  X rides the
        # scalar-engine HWDGE ring so it doesn't queue ahead of W's
        # sync-engine ring (per-engine FIFO).
        x_sb = const_pool.tile([128, S_TILES, XDIM], FP8, tag="x")
        bounds = X_BOUNDS
        for i in range(len(bounds) - 1):
            lo, hi = bounds[i], bounds[i + 1]
            nc.scalar.dma_start(x_sb[:, lo:hi, :],
                                x_d[:, lo * XDIM:hi * XDIM])

        def epilogue(t, agg_ps):
            # LN -> transpose -> @ proj_w.T -> residual
            agg = ep_pool.tile([128, DIM], F32, tag="agg_sb")
            rowsum = ep_pool.tile([128, 1], F32, tag="rowsum")
            nc.scalar.activation(agg[:], agg_ps[:], AF.Copy,
                                 accum_out=rowsum[:])
            mean = ep_pool.tile([128, 1], F32, tag="mean")
            nc.scalar.mul(mean[:], rowsum[:], 1.0 / DIM)
            cent = ep_pool.tile([128, DIM], F32, tag="cent")
            nc.vector.tensor_scalar(cent[:], agg[:], mean[:], None,
                                    AluOpType.subtract)
            sq = ep_pool.tile([128, DIM], F32, tag="sq")
            sumsq = ep_pool.tile([128, 1], F32, tag="sumsq")
            nc.scalar.activation(sq[:], cent[:], AF.Square,
                                 accum_out=sumsq[:])
            std = ep_pool.tile([128, 1], F32, tag="std")
            nc.scalar.activation(std[:], sumsq[:], AF.Sqrt, bias=epsb[:],
                                 scale=1.0 / DIM)
            rstd = ep_pool.tile([128, 1], F32, tag="rstd")
            nc.vector.reciprocal(rstd[:], std[:])
            ln = ep_pool.tile([128, DIM], BF16, tag="ln")
            nc.vector.tensor_scalar(ln[:], cent[:], rstd[:], None,
                                    AluOpType.mult)

            y_ps = ps_y.tile([128, DIM], F32, tag="y")
            for k in range(2):
                tr_ps = ps_tr.tile([128, 128], BF16, tag="tr")
                nc.tensor.transpose(tr_ps[:], ln[:, k * 128:(k + 1) * 128],
                                    ident[:])
                lnT = ep_pool.tile([128, 128], BF16, tag="lnT")
                nc.scalar.copy(lnT[:], tr_ps[:])
                nc.tensor.matmul(y_ps[:], lnT[:], pwt_sb[:, k, :],
                                 start=(k == 0), stop=(k == 1))

            rel_t = ep_pool.tile([128, DIM], F32, tag="rel")
            nc.scalar.dma_start(rel_t[:], relsl_d[t * 128:(t + 1) * 128, :])
            delta = ep_pool.tile([128, DIM], F32, tag="delta")
            nc.vector.tensor_scalar(delta[:], y_ps[:], ALPHA, None,
                                    AluOpType.mult)
            out_t = ep_pool.tile([128, DIM], F32, tag="out")
            nc.vector.tensor_tensor(out_t[:], delta[:], rel_t[:],
                                    AluOpType.add)
            # scalar ring: keep the W (sync) FIFO free of store interleave
            nc.scalar.dma_start(out_d[t * 128:(t + 1) * 128, :], out_t[:])

        # software-pipelined: tile t's block MMs are emitted before tile
        # t-1's epilogue, so the PE (strict program order) never stalls on
        # the previous tile's LN chain -- it hides under the next MM run
        pending = []
        for t in range(N_TILES):
            wb = W_BOUNDS0 if t == 0 else W_BOUNDS
            w_t = wpool.tile([128, S_TILES, TILE], FP8, tag="w")
            for i in range(len(wb) - 1):
                lo, hi = wb[i], wb[i + 1]
                nc.sync.dma_start(w_t[:, lo:hi, :],
                                  w_d[t, :, lo * TILE:hi * TILE])

            agg_ps = ps_agg.tile([128, XDIM], F32, tag="agg")
            for j in range(S_PAIRS):
                nc.tensor.matmul(agg_ps[:], w_t[:, 2 * j:2 * j + 2, :],
                                 x_sb[:, 2 * j:2 * j + 2, :],
                                 start=(j == 0), stop=(j == S_PAIRS - 1),
                                 perf_mode=mybir.MatmulPerfMode.DoubleRow)
            pending.append((t, agg_ps))
            if len(pending) > 1:
                epilogue(*pending.pop(0))
        for p in pending:
            epilogue(*p)

    nc.compile()
    return nc


def _prep(rel_embed, rel_edge_index, rel_edge_weight, proj_w):
    """Host-side sharding/layout: scatter edges into dense per-(dst tile,
    src tile) weight blocks; lay out rel_embed for SBUF residency."""
    src = np.asarray(rel_edge_index[0], dtype=np.int64)
    dst = np.asarray(rel_edge_index[1], dtype=np.int64)
    w = np.asarray(rel_edge_weight, dtype=np.float32)
    rel = np.asarray(rel_embed, dtype=np.float32)
    pw = np.asarray(proj_w, dtype=np.float32)

    core = dst // NODES_PER_CORE
    drel = dst - core * NODES_PER_CORE
    t = drel // TILE
    d = drel % TILE
    s = src // TILE
    p = src % TILE
    # flat index inside one core's [N_TILES, S_TILES, 128, 128] block array
    flat = ((t * S_TILES + s) * TILE + p) * TILE + d
    blk_sz = N_TILES * S_TILES * TILE * TILE

    w_dev = np.empty((N_CORES, N_TILES, 128, S_TILES * TILE),
                     dtype=ml_dtypes.float8_e4m3)
    for c in range(N_CORES):
        m = core == c
        wc = np.bincount(flat[m], weights=w[m], minlength=blk_sz)
        wc = wc.reshape(N_TILES, S_TILES, TILE, TILE).astype(np.float32)
        # -> [t, p(src), s*128+d(dst)] so the SBUF tile is partition=src
        w_dev[c] = wc.transpose(0, 2, 1, 3).reshape(
            N_TILES, 128, S_TILES * TILE)

    rel16 = rel.astype(ml_dtypes.float8_e4m3)
    rel16_pad = np.zeros((S_TILES * TILE, XDIM), dtype=ml_dtypes.float8_e4m3)
    rel16_pad[:N_NODES, :DIM] = rel16
    x_dev = np.ascontiguousarray(
        rel16_pad.reshape(S_TILES, TILE, XDIM).transpose(1, 0, 2).reshape(
            128, S_TILES * XDIM))

    relslice = np.zeros((N_CORES, OUT_ROWS, DIM), dtype=np.float32)
    for c in range(N_CORES):
        relslice[c, :NODES_PER_CORE] = rel[c * NODES_PER_CORE:
                                           (c + 1) * NODES_PER_CORE]
    pwt = pw.T.astype(ml_dtypes.bfloat16)  # [f, o]
    pwt_dev = np.ascontiguousarray(
        pwt.reshape(2, 128, DIM).transpose(1, 0, 2).reshape(128, 2 * DIM))

    in_maps = []
    for c in range(N_CORES):
        in_maps.append({
            "x": x_dev,
            "wblk": w_dev[c],
            "relslice": relslice[c],
            "projwT": pwt_dev,
        })
    return in_maps


def kernel(rel_embed, rel_edge_index, rel_edge_weight, proj_w,
           _trace=False):
    in_maps = _prep(rel_embed, rel_edge_index, rel_edge_weight, proj_w)
    nc = _CACHE.get("nc")
    if nc is None:
        nc = _build()
        _CACHE["nc"] = nc
    res = run_bass_kernel_spmd(nc, in_maps, core_ids=list(range(N_CORES)),
                               trace=_trace)
    out = np.concatenate(
        [res.results[c]["out"][:NODES_PER_CORE] for c in range(N_CORES)],
        axis=0)
    if _trace:
        kernel.last_results = res
    return out.astype(np.float32)

